# Initial kernel scaffold
#
"""MoE genre-gate router kernel for Trainium2 (8 NeuronCores, SPMD).

Computes, for x:[B,S,D], genre_emb:[B,DG], Wx:[D,E], Wg:[DG,E], b:[E]:
    logits = x @ Wx + (genre_emb @ Wg)[:,None,:] + b
    probs  = softmax(logits); top2 = top_k(probs, 2)
    gates  = scatter(top2_vals / sum(top2_vals));  topk_idx
Returns (gates [B,S,E] f32, topk_idx [B,S,K] int32).

Sharding: data/sequence parallel. Core c handles batch c//2, seq half c%2
(4096 tokens each). Router weights replicated. No cross-core communication.

Per-core kernel: tokens are mapped token = p*32 + g (partition-major) so all
DMAs are contiguous per partition. Per 512-token block:
  - one 2 MiB x DMA  [128, 4, 1024]
  - PE transposes each [128tok, 128d] chunk (fp32 exact), ACT/DVE copy
    PSUM->SBUF, then fp32 matmuls lhsT=xT_chunk rhs=Wx_chunk accumulate
    logits [128, 64] per token-tile in PSUM
  - genre+bias logits precomputed once, broadcast to 128 partitions via a
    ones-column matmul, added during the PSUM->SBUF logits copy
  - top-2 via DVE max/max_index (tie order matches lax.top_k), renorm
    weights via exp + reciprocal (softmax Z cancels algebraically),
    gates scattered with fused tensor_scalar (is_ge, mult) ops
"""

import numpy as np

B, S, D = 4, 8192, 1024
DG = 256
E, K = 64, 2
N_CORES = 8
P = 128
T_SHARD = B * S // N_CORES  # 4096 tokens per core

_CACHE = {}


def _build(T=T_SHARD, act_chunks=6):
    """Build + compile the per-core Bass program. T = tokens per shard."""
    from contextlib import ExitStack

    import concourse.mybir as mybir
    import concourse.tile as tile
    from concourse import bacc
    from concourse.masks import make_identity

    f32 = mybir.dt.float32
    Alu = mybir.AluOpType
    Act = mybir.ActivationFunctionType

    QB = 4            # token-tiles of 128 per block
    BLK = P * QB      # 512 tokens per block
    NBLK = T // BLK
    GT = T // P       # tokens per partition
    DC = D // P       # 8 contraction chunks
    assert T % BLK == 0

    nc = bacc.Bacc("TRN2", target_bir_lowering=False, debug=False)

    x_d = nc.dram_tensor("x_shard", [T, D], f32, kind="ExternalInput").ap()
    genre_d = nc.dram_tensor("genre_shard", [DG], f32, kind="ExternalInput").ap()
    wx_d = nc.dram_tensor("Wx", [D, E], f32, kind="ExternalInput").ap()
    wg_d = nc.dram_tensor("Wg", [DG, E], f32, kind="ExternalInput").ap()
    b_d = nc.dram_tensor("b", [E], f32, kind="ExternalInput").ap()
    gates_d = nc.dram_tensor("gates_shard", [T, E], f32, kind="ExternalOutput").ap()
    idx_d = nc.dram_tensor("idx_shard", [T, K], mybir.dt.int32, kind="ExternalOutput").ap()

    # token t of the shard lives at (partition p, slot g): t = p*GT + g
    xv = x_d.rearrange("(p g) d -> p g d", p=P)        # [128, GT, 1024]
    gv = gates_d.rearrange("(p g) e -> p g e", p=P)    # [128, GT, 64]
    iv = idx_d.rearrange("(p g) k -> p g k", p=P)      # [128, GT, 2]

    with tile.TileContext(nc) as tc, ExitStack() as ctx:
        const = ctx.enter_context(tc.tile_pool(name="const", bufs=1))

        identity = const.tile([P, P], f32)
        make_identity(nc, identity)

        wx_sb = const.tile([P, DC, E], f32)
        nc.sync.dma_start(wx_sb, wx_d.rearrange("(c p) e -> p c e", p=P))
        wg_sb = const.tile([P, DG // P, E], f32)
        nc.sync.dma_start(wg_sb, wg_d.rearrange("(c p) e -> p c e", p=P))
        genre_sb = const.tile([P, DG // P], f32)
        nc.sync.dma_start(genre_sb, genre_d.rearrange("(c p) -> p c", p=P))
        b_row = const.tile([1, E], f32)
        nc.sync.dma_start(b_row, b_d[None, :])
        ones_col = const.tile([1, P], f32)
        nc.vector.memset(ones_col, 1.0)

        # genre logits g_row[1,E] = genre @ Wg + b, then broadcast to all
        # 128 partitions with a ones-column matmul -> g_bcast [128, E]
        g_bcast = const.tile([P, E], f32)
        with tc.tile_pool(name="psum_pre", bufs=1, space="PSUM") as psum_pre:
            g_ps = psum_pre.tile([1, E], f32)
            for c in range(DG // P):
                nc.tensor.matmul(
                    g_ps, genre_sb[:, c : c + 1], wg_sb[:, c, :],
                    start=(c == 0), stop=(c == DG // P - 1),
                )
            g_row = const.tile([1, E], f32)
            nc.vector.tensor_add(g_row, g_ps, b_row)
            gb_ps = psum_pre.tile([P, E], f32)
            nc.tensor.matmul(gb_ps, ones_col, g_row, start=True, stop=True)
            nc.vector.tensor_copy(g_bcast, gb_ps)

        idx_all = const.tile([P, GT, K], mybir.dt.int32)

        xpool = ctx.enter_context(tc.tile_pool(name="xblk", bufs=2))
        xtpool = ctx.enter_context(tc.tile_pool(name="xt", bufs=3))
        lpool = ctx.enter_context(tc.tile_pool(name="lsb", bufs=2))
        gpool = ctx.enter_context(tc.tile_pool(name="gatep", bufs=2))
        vpool = ctx.enter_context(tc.tile_pool(name="valp", bufs=2))
        psum_xt = ctx.enter_context(tc.tile_pool(name="psum_xt", bufs=3, space="PSUM"))
        psum_l = ctx.enter_context(tc.tile_pool(name="psum_l", bufs=2, space="PSUM"))

        for j in range(NBLK):
            x_blk = xpool.tile([P, QB, D], f32, name="x_blk")
            nc.sync.dma_start(x_blk, xv[:, j * QB : (j + 1) * QB, :])

            lg_ps = psum_l.tile([P, QB, E], f32, name="lg_ps")
            for q in range(QB):
                # transpose x tile: 8 chunks [128tok,128d] -> [128d,128tok]
                xt_ps = psum_xt.tile([P, DC, P], f32, name="xt_ps")
                for c in range(DC):
                    nc.tensor.transpose(
                        xt_ps[:, c, :], x_blk[:, q, c * P : (c + 1) * P], identity
                    )
                xt_sb = xtpool.tile([P, DC, P], f32, name="xt_sb")
                nc.scalar.copy(xt_sb[:, :act_chunks, :], xt_ps[:, :act_chunks, :])
                nc.vector.tensor_copy(xt_sb[:, act_chunks:, :], xt_ps[:, act_chunks:, :])
                # logits[tok, e] += xT_c.T @ Wx_c
                for c in range(DC):
                    nc.tensor.matmul(
                        lg_ps[:, q, :], xt_sb[:, c, :], wx_sb[:, c, :],
                        start=(c == 0), stop=(c == DC - 1),
                    )

            # PSUM -> SBUF with fused genre+bias add
            l_sb = lpool.tile([P, QB, E], f32, name="l_sb")
            nc.vector.tensor_tensor(
                l_sb, lg_ps, g_bcast[:, None, :].to_broadcast((P, QB, E)), Alu.add
            )

            # top-2 per token (one token per partition-row per q slot)
            vals8 = vpool.tile([P, QB, 8], f32, name="vals8")
            idx8 = vpool.tile([P, QB, 8], mybir.dt.uint32, name="idx8")
            for q in range(QB):
                nc.vector.max(out=vals8[:, q, :], in_=l_sb[:, q, :])
                nc.vector.max_index(
                    out=idx8[:, q, :], in_max=vals8[:, q, :], in_values=l_sb[:, q, :]
                )

            # renormalized top-2 weights: w1 = 1/(1+p2), w2 = p2/(1+p2),
            # p2 = exp(v2 - v1)   (softmax denominator cancels)
            dv = vpool.tile([P, QB], f32, name="dv")
            p2 = vpool.tile([P, QB], f32, name="p2")
            sinv = vpool.tile([P, QB], f32, name="sinv")
            w2 = vpool.tile([P, QB], f32, name="w2")
            wd = vpool.tile([P, QB], f32, name="wd")
            nc.vector.tensor_tensor(dv, vals8[:, :, 1], vals8[:, :, 0], Alu.subtract)
            nc.scalar.activation(p2, dv, Act.Exp)
            nc.vector.tensor_scalar_add(w2, p2, 1.0)
            nc.vector.reciprocal(sinv, w2)                      # 1/(1+p2) = w1
            nc.vector.tensor_mul(w2, p2, sinv)                  # w2
            nc.vector.tensor_tensor(wd, sinv, w2, Alu.subtract)  # w1 - w2

            # gates = (l >= v2)*w2 + (l >= v1)*(w1 - w2)
            gates_sb = gpool.tile([P, QB, E], f32, name="gates_sb")
            tmpg = gpool.tile([P, QB, E], f32, name="tmpg")
            for q in range(QB):
                nc.vector.tensor_scalar(
                    gates_sb[:, q, :], l_sb[:, q, :],
                    vals8[:, q, 1:2], w2[:, q : q + 1], Alu.is_ge, Alu.mult,
                )
                nc.vector.tensor_scalar(
                    tmpg[:, q, :], l_sb[:, q, :],
                    vals8[:, q, 0:1], wd[:, q : q + 1], Alu.is_ge, Alu.mult,
                )
            nc.vector.tensor_add(gates_sb, gates_sb, tmpg)

            nc.vector.tensor_copy(idx_all[:, j * QB : (j + 1) * QB, :], idx8[:, :, :K])
            nc.scalar.dma_start(gv[:, j * QB : (j + 1) * QB, :], gates_sb)

        nc.scalar.dma_start(iv, idx_all)

    nc.compile()
    return nc


def _get_nc():
    if "nc" not in _CACHE:
        _CACHE["nc"] = _build()
    return _CACHE["nc"]


def kernel(x, genre_emb, Wx, Wg, b):
    from concourse.bass_utils import run_bass_kernel_spmd

    x = np.asarray(x, dtype=np.float32)
    genre_emb = np.asarray(genre_emb, dtype=np.float32)
    Wx = np.asarray(Wx, dtype=np.float32)
    Wg = np.asarray(Wg, dtype=np.float32)
    b = np.asarray(b, dtype=np.float32)

    nc = _get_nc()

    half = S // 2
    in_maps = []
    for c in range(N_CORES):
        bi, h = divmod(c, 2)
        in_maps.append(
            {
                "x_shard": np.ascontiguousarray(x[bi, h * half : (h + 1) * half, :]),
                "genre_shard": genre_emb[bi],
                "Wx": Wx,
                "Wg": Wg,
                "b": b,
            }
        )

    res = run_bass_kernel_spmd(nc, in_maps, list(range(N_CORES)))
    _CACHE["last_results"] = res

    gates = np.empty((B, S, E), dtype=np.float32)
    idx = np.empty((B, S, K), dtype=np.int32)
    for c in range(N_CORES):
        bi, h = divmod(c, 2)
        gates[bi, h * half : (h + 1) * half, :] = res.results[c]["gates_shard"]
        idx[bi, h * half : (h + 1) * half, :] = res.results[c]["idx_shard"]
    return gates, idx


# revision 5
# speedup vs baseline: 1.0049x; 1.0049x over previous
"""MoE genre-gate router kernel for Trainium2 (8 NeuronCores, SPMD).

Computes, for x:[B,S,D], genre_emb:[B,DG], Wx:[D,E], Wg:[DG,E], b:[E]:
    logits = x @ Wx + (genre_emb @ Wg)[:,None,:] + b
    probs  = softmax(logits); top2 = top_k(probs, 2)
    gates  = scatter(top2_vals / sum(top2_vals));  topk_idx
Returns (gates [B,S,E] f32, topk_idx [B,S,K] int32).

Sharding: data/sequence parallel. Core c handles batch c//2, seq half c%2
(4096 tokens each). Router weights replicated. No cross-core communication.

Per-core kernel: tokens are mapped token = p*32 + g (partition-major) so all
DMAs are contiguous per partition. Per 512-token block:
  - one 2 MiB x DMA  [128, 4, 1024]
  - PE transposes each [128tok, 128d] chunk (fp32 exact), ACT/DVE copy
    PSUM->SBUF, then fp32 matmuls lhsT=xT_chunk rhs=Wx_chunk accumulate
    logits [128, 64] per token-tile in PSUM
  - genre+bias logits precomputed once, broadcast to 128 partitions via a
    ones-column matmul, added during the PSUM->SBUF logits copy
  - top-2 via DVE max/max_index (tie order matches lax.top_k), renorm
    weights via exp + reciprocal (softmax Z cancels algebraically),
    gates scattered with fused tensor_scalar (is_ge, mult) ops
"""

import numpy as np

B, S, D = 4, 8192, 1024
DG = 256
E, K = 64, 2
N_CORES = 8
P = 128
T_SHARD = B * S // N_CORES  # 4096 tokens per core

_CACHE = {}


def _build(T=T_SHARD, act_chunks=6, reps=1):
    """Build + compile the per-core Bass program. T = tokens per shard.

    reps>1 repeats the whole main loop (idempotent) for HW timing via the
    (t_R - t_1)/(R-1) delta method — no NTFF profiling in this container.
    """
    from contextlib import ExitStack

    import concourse.mybir as mybir
    import concourse.tile as tile
    from concourse import bacc
    from concourse.masks import make_identity

    f32 = mybir.dt.float32
    Alu = mybir.AluOpType
    Act = mybir.ActivationFunctionType

    QB = 4            # token-tiles of 128 per block
    BLK = P * QB      # 512 tokens per block
    NBLK = T // BLK
    GT = T // P       # tokens per partition
    DC = D // P       # 8 contraction chunks
    assert T % BLK == 0

    nc = bacc.Bacc("TRN2", target_bir_lowering=False, debug=False)

    x_d = nc.dram_tensor("x_shard", [T, D], f32, kind="ExternalInput").ap()
    genre_d = nc.dram_tensor("genre_shard", [DG], f32, kind="ExternalInput").ap()
    wx_d = nc.dram_tensor("Wx", [D, E], f32, kind="ExternalInput").ap()
    wg_d = nc.dram_tensor("Wg", [DG, E], f32, kind="ExternalInput").ap()
    b_d = nc.dram_tensor("b", [E], f32, kind="ExternalInput").ap()
    gates_d = nc.dram_tensor("gates_shard", [T, E], f32, kind="ExternalOutput").ap()
    idx_d = nc.dram_tensor("idx_shard", [T, K], mybir.dt.int32, kind="ExternalOutput").ap()

    # token t of the shard lives at (partition p, slot g): t = p*GT + g
    xv = x_d.rearrange("(p g) d -> p g d", p=P)        # [128, GT, 1024]
    gv = gates_d.rearrange("(p g) e -> p g e", p=P)    # [128, GT, 64]
    iv = idx_d.rearrange("(p g) k -> p g k", p=P)      # [128, GT, 2]

    with tile.TileContext(nc) as tc, ExitStack() as ctx:
        const = ctx.enter_context(tc.tile_pool(name="const", bufs=1))

        identity = const.tile([P, P], f32)
        make_identity(nc, identity)

        wx_sb = const.tile([P, DC, E], f32)
        nc.sync.dma_start(wx_sb, wx_d.rearrange("(c p) e -> p c e", p=P))
        wg_sb = const.tile([P, DG // P, E], f32)
        nc.sync.dma_start(wg_sb, wg_d.rearrange("(c p) e -> p c e", p=P))
        genre_sb = const.tile([P, DG // P], f32)
        nc.sync.dma_start(genre_sb, genre_d.rearrange("(c p) -> p c", p=P))
        b_row = const.tile([1, E], f32)
        nc.sync.dma_start(b_row, b_d[None, :])
        ones_col = const.tile([1, P], f32)
        nc.vector.memset(ones_col, 1.0)

        # genre logits g_row[1,E] = genre @ Wg + b, then broadcast to all
        # 128 partitions with a ones-column matmul -> g_bcast [128, E]
        g_bcast = const.tile([P, E], f32)
        with tc.tile_pool(name="psum_pre", bufs=1, space="PSUM") as psum_pre:
            g_ps = psum_pre.tile([1, E], f32)
            for c in range(DG // P):
                nc.tensor.matmul(
                    g_ps, genre_sb[:, c : c + 1], wg_sb[:, c, :],
                    start=(c == 0), stop=(c == DG // P - 1),
                )
            g_row = const.tile([1, E], f32)
            nc.vector.tensor_add(g_row, g_ps, b_row)
            gb_ps = psum_pre.tile([P, E], f32)
            nc.tensor.matmul(gb_ps, ones_col, g_row, start=True, stop=True)
            nc.vector.tensor_copy(g_bcast, gb_ps)

        idx_all = const.tile([P, GT, K], mybir.dt.int32)

        xpool = ctx.enter_context(tc.tile_pool(name="xblk", bufs=2))
        xtpool = ctx.enter_context(tc.tile_pool(name="xt", bufs=3))
        lpool = ctx.enter_context(tc.tile_pool(name="lsb", bufs=2))
        gpool = ctx.enter_context(tc.tile_pool(name="gatep", bufs=2))
        vpool = ctx.enter_context(tc.tile_pool(name="valp", bufs=2))
        psum_xt = ctx.enter_context(tc.tile_pool(name="psum_xt", bufs=3, space="PSUM"))
        psum_l = ctx.enter_context(tc.tile_pool(name="psum_l", bufs=2, space="PSUM"))

        for j in range(NBLK * reps):
            j = j % NBLK
            x_blk = xpool.tile([P, QB, D], f32, name="x_blk")
            nc.sync.dma_start(x_blk, xv[:, j * QB : (j + 1) * QB, :])

            lg_ps = psum_l.tile([P, QB, E], f32, name="lg_ps")
            for q in range(QB):
                # transpose x tile: 8 chunks [128tok,128d] -> [128d,128tok]
                xt_ps = psum_xt.tile([P, DC, P], f32, name="xt_ps")
                for c in range(DC):
                    nc.tensor.transpose(
                        xt_ps[:, c, :], x_blk[:, q, c * P : (c + 1) * P], identity
                    )
                xt_sb = xtpool.tile([P, DC, P], f32, name="xt_sb")
                nc.scalar.copy(xt_sb[:, :act_chunks, :], xt_ps[:, :act_chunks, :])
                nc.vector.tensor_copy(xt_sb[:, act_chunks:, :], xt_ps[:, act_chunks:, :])
                # logits[tok, e] += xT_c.T @ Wx_c
                for c in range(DC):
                    nc.tensor.matmul(
                        lg_ps[:, q, :], xt_sb[:, c, :], wx_sb[:, c, :],
                        start=(c == 0), stop=(c == DC - 1),
                    )

            # PSUM -> SBUF with fused genre+bias add
            l_sb = lpool.tile([P, QB, E], f32, name="l_sb")
            nc.vector.tensor_tensor(
                l_sb, lg_ps, g_bcast[:, None, :].to_broadcast((P, QB, E)), Alu.add
            )

            # top-2 per token (one token per partition-row per q slot)
            vals8 = vpool.tile([P, QB, 8], f32, name="vals8")
            idx8 = vpool.tile([P, QB, 8], mybir.dt.uint32, name="idx8")
            for q in range(QB):
                nc.vector.max(out=vals8[:, q, :], in_=l_sb[:, q, :])
                nc.vector.max_index(
                    out=idx8[:, q, :], in_max=vals8[:, q, :], in_values=l_sb[:, q, :]
                )

            # renormalized top-2 weights: w1 = 1/(1+p2), w2 = p2/(1+p2),
            # p2 = exp(v2 - v1)   (softmax denominator cancels)
            dv = vpool.tile([P, QB], f32, name="dv")
            p2 = vpool.tile([P, QB], f32, name="p2")
            sinv = vpool.tile([P, QB], f32, name="sinv")
            w2 = vpool.tile([P, QB], f32, name="w2")
            wd = vpool.tile([P, QB], f32, name="wd")
            nc.vector.tensor_tensor(dv, vals8[:, :, 1], vals8[:, :, 0], Alu.subtract)
            nc.scalar.activation(p2, dv, Act.Exp)
            nc.vector.tensor_scalar_add(w2, p2, 1.0)
            nc.vector.reciprocal(sinv, w2)                      # 1/(1+p2) = w1
            nc.vector.tensor_mul(w2, p2, sinv)                  # w2
            nc.vector.tensor_tensor(wd, sinv, w2, Alu.subtract)  # w1 - w2

            # gates = (l >= v2)*w2 + (l >= v1)*(w1 - w2)
            gates_sb = gpool.tile([P, QB, E], f32, name="gates_sb")
            tmpg = gpool.tile([P, QB, E], f32, name="tmpg")
            for q in range(QB):
                nc.vector.tensor_scalar(
                    gates_sb[:, q, :], l_sb[:, q, :],
                    vals8[:, q, 1:2], w2[:, q : q + 1], Alu.is_ge, Alu.mult,
                )
                nc.vector.tensor_scalar(
                    tmpg[:, q, :], l_sb[:, q, :],
                    vals8[:, q, 0:1], wd[:, q : q + 1], Alu.is_ge, Alu.mult,
                )
            nc.vector.tensor_add(gates_sb, gates_sb, tmpg)

            nc.vector.tensor_copy(idx_all[:, j * QB : (j + 1) * QB, :], idx8[:, :, :K])
            nc.scalar.dma_start(gv[:, j * QB : (j + 1) * QB, :], gates_sb)

        nc.scalar.dma_start(iv, idx_all)

    nc.compile()
    return nc


def _get_nc():
    if "nc" not in _CACHE:
        _CACHE["nc"] = _build()
    return _CACHE["nc"]


def _make_in_maps(x, genre_emb, Wx, Wg, b):
    half = S // 2
    in_maps = []
    for c in range(N_CORES):
        bi, h = divmod(c, 2)
        in_maps.append(
            {
                "x_shard": np.ascontiguousarray(x[bi, h * half : (h + 1) * half, :]),
                "genre_shard": np.ascontiguousarray(genre_emb[bi]),
                "Wx": Wx,
                "Wg": Wg,
                "b": b,
            }
        )
    return in_maps


def kernel(x, genre_emb, Wx, Wg, b):
    from concourse.bass_utils import run_bass_kernel_spmd

    x = np.asarray(x, dtype=np.float32)
    genre_emb = np.asarray(genre_emb, dtype=np.float32)
    Wx = np.asarray(Wx, dtype=np.float32)
    Wg = np.asarray(Wg, dtype=np.float32)
    b = np.asarray(b, dtype=np.float32)

    nc = _get_nc()
    in_maps = _make_in_maps(x, genre_emb, Wx, Wg, b)
    res = run_bass_kernel_spmd(nc, in_maps, list(range(N_CORES)))
    _CACHE["last_results"] = res

    half = S // 2
    gates = np.empty((B, S, E), dtype=np.float32)
    idx = np.empty((B, S, K), dtype=np.int32)
    for c in range(N_CORES):
        bi, h = divmod(c, 2)
        gates[bi, h * half : (h + 1) * half, :] = res.results[c]["gates_shard"]
        idx[bi, h * half : (h + 1) * half, :] = res.results[c]["idx_shard"]
    return gates, idx


# revision 8
# speedup vs baseline: 5.6303x; 5.6029x over previous
"""MoE genre-gate router kernel for Trainium2 (8 NeuronCores, SPMD).

Computes, for x:[B,S,D], genre_emb:[B,DG], Wx:[D,E], Wg:[DG,E], b:[E]:
    logits = x @ Wx + (genre_emb @ Wg)[:,None,:] + b
    probs  = softmax(logits); top2 = top_k(probs, 2)
    gates  = scatter(top2_vals / sum(top2_vals));  topk_idx
Returns (gates [B,S,E] f32, topk_idx [B,S,K] int32).

Sharding: data/sequence parallel. Core c handles batch c//2, seq half c%2
(4096 tokens each). Router weights replicated. No cross-core communication.

Per-core kernel: tokens are mapped token = p*32 + g (partition-major) so all
DMAs are contiguous per partition. Per 512-token block:
  - one 2 MiB x DMA  [128, 4, 1024]
  - PE transposes each [128tok, 128d] chunk (fp32 exact), ACT/DVE copy
    PSUM->SBUF, then fp32 matmuls lhsT=xT_chunk rhs=Wx_chunk accumulate
    logits [128, 64] per token-tile in PSUM
  - genre+bias logits precomputed once, broadcast to 128 partitions via a
    ones-column matmul, added during the PSUM->SBUF logits copy
  - top-2 via DVE max/max_index (tie order matches lax.top_k), renorm
    weights via exp + reciprocal (softmax Z cancels algebraically),
    gates scattered with fused tensor_scalar (is_ge, mult) ops
"""

import numpy as np

B, S, D = 4, 8192, 1024
DG = 256
E, K = 64, 2
N_CORES = 8
P = 128
T_SHARD = B * S // N_CORES  # 4096 tokens per core

_CACHE = {}


def _build(T=T_SHARD, act_chunks=6, reps=1, mode="full"):
    """Build + compile the per-core Bass program. T = tokens per shard.

    reps>1 repeats the whole main loop (idempotent) for HW timing via the
    (t_R - t_1)/(R-1) delta method — no NTFF profiling in this container.
    mode: "full" | "dma" (x loads only) | "nogate" (skip top-k math) |
          "nomm" (skip logit matmuls) | "notrans" (skip transposes+copies) —
          ablation variants for bottleneck attribution; outputs invalid.
    """
    from contextlib import ExitStack

    import concourse.mybir as mybir
    import concourse.tile as tile
    from concourse import bacc
    from concourse.masks import make_identity

    f32 = mybir.dt.float32
    Alu = mybir.AluOpType
    Act = mybir.ActivationFunctionType

    QB = 4            # token-tiles of 128 per block
    BLK = P * QB      # 512 tokens per block
    NBLK = T // BLK
    GT = T // P       # tokens per partition
    DC = D // P       # 8 contraction chunks
    assert T % BLK == 0

    nc = bacc.Bacc("TRN2", target_bir_lowering=False, debug=False)

    x_d = nc.dram_tensor("x_shard", [T, D], f32, kind="ExternalInput").ap()
    genre_d = nc.dram_tensor("genre_shard", [DG], f32, kind="ExternalInput").ap()
    wx_d = nc.dram_tensor("Wx", [D, E], f32, kind="ExternalInput").ap()
    wg_d = nc.dram_tensor("Wg", [DG, E], f32, kind="ExternalInput").ap()
    b_d = nc.dram_tensor("b", [E], f32, kind="ExternalInput").ap()
    gates_d = nc.dram_tensor("gates_shard", [T, E], f32, kind="ExternalOutput").ap()
    idx_d = nc.dram_tensor("idx_shard", [T, K], mybir.dt.int32, kind="ExternalOutput").ap()

    # token t of the shard lives at (partition p, slot g): t = p*GT + g
    xv = x_d.rearrange("(p g) d -> p g d", p=P)        # [128, GT, 1024]
    gv = gates_d.rearrange("(p g) e -> p g e", p=P)    # [128, GT, 64]
    iv = idx_d.rearrange("(p g) k -> p g k", p=P)      # [128, GT, 2]

    with tile.TileContext(nc) as tc, ExitStack() as ctx:
        const = ctx.enter_context(tc.tile_pool(name="const", bufs=1))

        identity = const.tile([P, P], f32)
        make_identity(nc, identity)

        wx_sb = const.tile([P, DC, E], f32)
        nc.sync.dma_start(wx_sb, wx_d.rearrange("(c p) e -> p c e", p=P))
        wg_sb = const.tile([P, DG // P, E], f32)
        nc.sync.dma_start(wg_sb, wg_d.rearrange("(c p) e -> p c e", p=P))
        genre_sb = const.tile([P, DG // P], f32)
        nc.sync.dma_start(genre_sb, genre_d.rearrange("(c p) -> p c", p=P))
        b_row = const.tile([1, E], f32)
        nc.sync.dma_start(b_row, b_d[None, :])
        ones_col = const.tile([1, P], f32)
        nc.vector.memset(ones_col, 1.0)

        # genre logits g_row[1,E] = genre @ Wg + b, then broadcast to all
        # 128 partitions with a ones-column matmul -> g_bcast [128, E]
        g_bcast = const.tile([P, E], f32)
        with tc.tile_pool(name="psum_pre", bufs=1, space="PSUM") as psum_pre:
            g_ps = psum_pre.tile([1, E], f32)
            for c in range(DG // P):
                nc.tensor.matmul(
                    g_ps, genre_sb[:, c : c + 1], wg_sb[:, c, :],
                    start=(c == 0), stop=(c == DG // P - 1),
                )
            g_row = const.tile([1, E], f32)
            nc.vector.tensor_add(g_row, g_ps, b_row)
            gb_ps = psum_pre.tile([P, E], f32)
            nc.tensor.matmul(gb_ps, ones_col, g_row, start=True, stop=True)
            nc.vector.tensor_copy(g_bcast, gb_ps)

        idx_all = const.tile([P, GT, K], mybir.dt.int32)
        if mode != "full":
            nc.vector.memset(idx_all, 0)

        xpool = ctx.enter_context(tc.tile_pool(name="xblk", bufs=2))
        xtpool = ctx.enter_context(tc.tile_pool(name="xt", bufs=3))
        lpool = ctx.enter_context(tc.tile_pool(name="lsb", bufs=2))
        gpool = ctx.enter_context(tc.tile_pool(name="gatep", bufs=2))
        vpool = ctx.enter_context(tc.tile_pool(name="valp", bufs=2))
        psum_xt = ctx.enter_context(tc.tile_pool(name="psum_xt", bufs=3, space="PSUM"))
        psum_l = ctx.enter_context(tc.tile_pool(name="psum_l", bufs=2, space="PSUM"))

        first_xt = {}
        for j in range(NBLK * reps):
            j = j % NBLK
            x_blk = xpool.tile([P, QB, D], f32, name="x_blk")
            nc.sync.dma_start(x_blk, xv[:, j * QB : (j + 1) * QB, :])
            if mode == "dma":
                continue

            lg_ps = psum_l.tile([P, QB, E], f32, name="lg_ps")
            for q in range(QB):
                if mode != "notrans":
                    # transpose x tile: 8 chunks [128tok,128d] -> [128d,128tok]
                    xt_ps = psum_xt.tile([P, DC, P], f32, name="xt_ps")
                    for c in range(DC):
                        nc.tensor.transpose(
                            xt_ps[:, c, :], x_blk[:, q, c * P : (c + 1) * P], identity
                        )
                    xt_sb = xtpool.tile([P, DC, P], f32, name="xt_sb")
                    nc.scalar.copy(xt_sb[:, :act_chunks, :], xt_ps[:, :act_chunks, :])
                    nc.vector.tensor_copy(
                        xt_sb[:, act_chunks:, :], xt_ps[:, act_chunks:, :]
                    )
                else:
                    # reuse one stale xt tile so MMs have an SBUF operand
                    if "t" not in first_xt:
                        first_xt["t"] = xtpool.tile([P, DC, P], f32, name="xt_sb")
                        nc.vector.memset(first_xt["t"], 0.5)
                    xt_sb = first_xt["t"]
                if mode == "nomm":
                    continue
                # logits[tok, e] += xT_c.T @ Wx_c
                for c in range(DC):
                    nc.tensor.matmul(
                        lg_ps[:, q, :], xt_sb[:, c, :], wx_sb[:, c, :],
                        start=(c == 0), stop=(c == DC - 1),
                    )

            if mode == "nomm":
                continue
            # PSUM -> SBUF with fused genre+bias add
            l_sb = lpool.tile([P, QB, E], f32, name="l_sb")
            nc.vector.tensor_tensor(
                l_sb, lg_ps, g_bcast[:, None, :].to_broadcast((P, QB, E)), Alu.add
            )
            if mode == "nogate":
                nc.scalar.dma_start(gv[:, j * QB : (j + 1) * QB, :], l_sb)
                continue

            # top-2 per token (one token per partition-row per q slot)
            vals8 = vpool.tile([P, QB, 8], f32, name="vals8")
            idx8 = vpool.tile([P, QB, 8], mybir.dt.uint32, name="idx8")
            for q in range(QB):
                nc.vector.max(out=vals8[:, q, :], in_=l_sb[:, q, :])
                nc.vector.max_index(
                    out=idx8[:, q, :], in_max=vals8[:, q, :], in_values=l_sb[:, q, :]
                )

            # renormalized top-2 weights: w1 = 1/(1+p2), w2 = p2/(1+p2),
            # p2 = exp(v2 - v1)   (softmax denominator cancels)
            dv = vpool.tile([P, QB], f32, name="dv")
            p2 = vpool.tile([P, QB], f32, name="p2")
            sinv = vpool.tile([P, QB], f32, name="sinv")
            w2 = vpool.tile([P, QB], f32, name="w2")
            wd = vpool.tile([P, QB], f32, name="wd")
            nc.vector.tensor_tensor(dv, vals8[:, :, 1], vals8[:, :, 0], Alu.subtract)
            nc.scalar.activation(p2, dv, Act.Exp)
            nc.vector.tensor_scalar_add(w2, p2, 1.0)
            nc.vector.reciprocal(sinv, w2)                      # 1/(1+p2) = w1
            nc.vector.tensor_mul(w2, p2, sinv)                  # w2
            nc.vector.tensor_tensor(wd, sinv, w2, Alu.subtract)  # w1 - w2

            # gates = (l >= v2)*w2 + (l >= v1)*(w1 - w2)
            gates_sb = gpool.tile([P, QB, E], f32, name="gates_sb")
            tmpg = gpool.tile([P, QB, E], f32, name="tmpg")
            for q in range(QB):
                nc.vector.tensor_scalar(
                    gates_sb[:, q, :], l_sb[:, q, :],
                    vals8[:, q, 1:2], w2[:, q : q + 1], Alu.is_ge, Alu.mult,
                )
                nc.vector.tensor_scalar(
                    tmpg[:, q, :], l_sb[:, q, :],
                    vals8[:, q, 0:1], wd[:, q : q + 1], Alu.is_ge, Alu.mult,
                )
            nc.vector.tensor_add(gates_sb, gates_sb, tmpg)

            nc.vector.tensor_copy(idx_all[:, j * QB : (j + 1) * QB, :], idx8[:, :, :K])
            nc.scalar.dma_start(gv[:, j * QB : (j + 1) * QB, :], gates_sb)

        nc.scalar.dma_start(iv, idx_all)

    nc.compile()
    return nc


def _get_nc():
    if "nc" not in _CACHE:
        _CACHE["nc"] = _build()
    return _CACHE["nc"]


def _make_in_maps(x, genre_emb, Wx, Wg, b):
    half = S // 2
    in_maps = []
    for c in range(N_CORES):
        bi, h = divmod(c, 2)
        in_maps.append(
            {
                "x_shard": np.ascontiguousarray(x[bi, h * half : (h + 1) * half, :]),
                "genre_shard": np.ascontiguousarray(genre_emb[bi]),
                "Wx": Wx,
                "Wg": Wg,
                "b": b,
            }
        )
    return in_maps


def kernel(x, genre_emb, Wx, Wg, b):
    from concourse.bass_utils import run_bass_kernel_spmd

    x = np.asarray(x, dtype=np.float32)
    genre_emb = np.asarray(genre_emb, dtype=np.float32)
    Wx = np.asarray(Wx, dtype=np.float32)
    Wg = np.asarray(Wg, dtype=np.float32)
    b = np.asarray(b, dtype=np.float32)

    nc = _get_nc()
    in_maps = _make_in_maps(x, genre_emb, Wx, Wg, b)
    res = run_bass_kernel_spmd(nc, in_maps, list(range(N_CORES)))
    _CACHE["last_results"] = res

    half = S // 2
    gates = np.empty((B, S, E), dtype=np.float32)
    idx = np.empty((B, S, K), dtype=np.int32)
    for c in range(N_CORES):
        bi, h = divmod(c, 2)
        gates[bi, h * half : (h + 1) * half, :] = res.results[c]["gates_shard"]
        idx[bi, h * half : (h + 1) * half, :] = res.results[c]["idx_shard"]
    return gates, idx
